# revision 2
# baseline (speedup 1.0000x reference)
"""Trainium2 Bass kernel v2 for nn_Detail_loss (histogram_binning).

Data-parallel over B=32 samples -> 8 cores x 4 samples. Per core/sample:
  1. 5x5 dilation: vertical via PE banded matmuls in f32r (labels are 0/1 so
     f32r is exact), horizontal via row-cumsum difference (scan).
  2. w = 255*(256/255)*M*img in one stt; idx/h16/lo via magic-constant
     floor tricks, all bf16 where possible (DVE 4x mode). Unmasked pixels
     fall in bin (0,0); fixed by subtracting (262144 - nmask) later.
  3. Histogram via 16x16 hi/lo one-hot bf16 planes and 8-column-grouped
     PE outer products into one [128,128] PSUM tile (diagonal 16x16 blocks
     summed afterwards).
  4. Otsu on the simplified criterion bv = m0^2/(w0+e) + (m1-m0)^2/(w1+e)
     + (tm-m1)^2/(w2+e) (same argmax as reference). Row terms broadcast to
     127 partitions via exact f32 PE ones-matmuls (no gpsimd broadcasts).
     First-max row-major tie-break via the -BIG flat-index encoding.
  5. MSE via the expansion sq = 0.75*N12 - 0.5*N1 - SG + SPP with
     ge-compares on w against W_k = K1*T_k (exact modulo ~ulp windows),
     N's from accum_out, SG = sum((ge1+ge2)*prd) via tensor_tensor_reduce,
     SPP = sum(prd^2*M) precomputed in the front-end.
Host: loss = mean over valid samples of sq/sm (np.float32 math).
"""

import os

import numpy as np

import concourse.bass as bass
import concourse.mybir as mybir
from concourse import bacc, bass_isa, tile
from concourse.bass_utils import run_bass_kernel_spmd

F32 = mybir.dt.float32
F32R = mybir.dt.float32r
BF16 = mybir.dt.bfloat16
OP = mybir.AluOpType
ACT = mybir.ActivationFunctionType
AX = mybir.AxisListType

B_PER_CORE = 4
H = 512
W = 512
NSLAB = 4
SLABW = 512
NBINS = 256
NT = 254
BIG = 4194304.0      # 2^22
MAGIC = 8388608.0    # 2^23
MAGICM05 = 8388607.5
EPS = 1e-8
NPIX = 262144.0      # 512*512

C_BIN = float(np.float32(256.0 / 255.0))
K1 = float(np.float32(255.0) * np.float32(256.0 / 255.0))  # w = K1*img*M
R254 = float(np.float32(1.0) / np.float32(254.0))

# engine per one-hot plane (32 = 16 hi then 16 lo)
PLANE_ENG = (["dve"] * 8 + ["pool"] * 4 + ["act"] * 4 +
             ["dve"] * 11 + ["pool"] * 4 + ["act"] * 1)
assert len(PLANE_ENG) == 32
SCAN_ENG = ["dve", "dve", "dve", "dve"]  # scan reads PSUM; gpsimd cannot
M_ENG = ["dve", "dve", "dve", "dve"]  # accum_out unsupported on Pool

# stats columns in [128, 64]
SM0 = 0     # 0..15   sm(b,s)
SPP0 = 16   # 16..31  spp(b,s)
SG0 = 32    # 32..47  sg(b,s)
N1C = 48    # 48..51  N1(b)
N12C = 52   # 52..55  N12(b)


def build_nc():
    nc = bacc.Bacc("TRN2", target_bir_lowering=False)

    lab_d = nc.dram_tensor("labels", [B_PER_CORE * H, W], F32, kind="ExternalInput")
    img_d = nc.dram_tensor("images", [B_PER_CORE * H, W], F32, kind="ExternalInput")
    prd_d = nc.dram_tensor("preds", [B_PER_CORE * H, W], F32, kind="ExternalInput")
    out_d = nc.dram_tensor("stats", [64, 1], F32, kind="ExternalOutput")
    dbg_d = nc.dram_tensor("dbg", [1, 16], F32, kind="ExternalOutput")

    with tile.TileContext(nc) as tc:
        _emit(nc, tc, lab_d, img_d, prd_d, out_d, dbg_d)
    nc.compile()
    return nc


def _sample_view(dram, b):
    return dram[512 * b:512 * (b + 1), :].rearrange("(s p) c -> p s c", p=128)


def _eng(nc, name):
    return {"dve": nc.vector, "pool": nc.gpsimd, "act": nc.scalar}[name]


def _emit(nc, tc, lab_d, img_d, prd_d, out_d, dbg_d):
    import contextlib
    ctx = contextlib.ExitStack()
    with ctx:
        const = ctx.enter_context(tc.tile_pool(name="const", bufs=1))
        lab_pool = ctx.enter_context(tc.tile_pool(name="lab", bufs=1))
        img_pool = ctx.enter_context(tc.tile_pool(name="img", bufs=2))
        prd_pool = ctx.enter_context(tc.tile_pool(name="prd", bufs=2))
        m_pool = ctx.enter_context(tc.tile_pool(name="mask", bufs=2))
        w_pool = ctx.enter_context(tc.tile_pool(name="w", bufs=2))
        scr_pool = ctx.enter_context(tc.tile_pool(name="scr", bufs=3))
        scra_pool = ctx.enter_context(tc.tile_pool(name="scra", bufs=1))
        ge_pool = ctx.enter_context(tc.tile_pool(name="ge", bufs=1))
        plA_pool = ctx.enter_context(tc.tile_pool(name="plA", bufs=2))
        plB_pool = ctx.enter_context(tc.tile_pool(name="plB", bufs=2))
        otsu_pool = ctx.enter_context(tc.tile_pool(name="otsu", bufs=1))
        stat_pool = ctx.enter_context(tc.tile_pool(name="stat", bufs=1))
        vpsum = ctx.enter_context(
            tc.tile_pool(name="vpsum", bufs=1, space=bass.MemorySpace.PSUM))
        hpsum = ctx.enter_context(
            tc.tile_pool(name="hpsum", bufs=2, space=bass.MemorySpace.PSUM))
        bpsum = ctx.enter_context(
            tc.tile_pool(name="bpsum", bufs=1, space=bass.MemorySpace.PSUM))
        spsum = ctx.enter_context(
            tc.tile_pool(name="spsum", bufs=1, space=bass.MemorySpace.PSUM))

        # ---------------- constants ----------------
        io_fp = const.tile([128, 128], mybir.dt.int32, tag="io_fp")   # f - p
        nc.gpsimd.iota(io_fp[:], pattern=[[1, 128]], base=0, channel_multiplier=-1)
        io_pf = const.tile([128, 128], mybir.dt.int32, tag="io_pf")   # p - f
        nc.gpsimd.iota(io_pf[:], pattern=[[-1, 128]], base=0, channel_multiplier=1)

        bv_band = const.tile([128, 128], BF16, tag="bv_band")
        btmp = const.tile([128, 128], F32, tag="btmp")
        nc.vector.tensor_scalar(btmp[:], io_fp[:], -2, None, OP.is_ge)
        nc.vector.scalar_tensor_tensor(bv_band[:], io_fp[:], 2, btmp[:], OP.is_le, OP.mult)
        up_band = const.tile([128, 128], BF16, tag="up_band")
        nc.vector.tensor_scalar(up_band[:], io_pf[:], 126, None, OP.is_ge)
        dn_band = const.tile([128, 128], BF16, tag="dn_band")
        nc.vector.tensor_scalar(dn_band[:], io_fp[:], 126, None, OP.is_ge)

        io256 = const.tile([1, 256], F32, tag="io256")     # 0..255
        nc.gpsimd.iota(io256[:], pattern=[[1, 256]], base=0, channel_multiplier=0,
                       allow_small_or_imprecise_dtypes=True)
        iot = const.tile([1, NT], F32, tag="iot")          # 0..253
        nc.gpsimd.iota(iot[:], pattern=[[1, NT]], base=0, channel_multiplier=0,
                       allow_small_or_imprecise_dtypes=True)
        iobig = const.tile([127, NT], F32, tag="iobig")    # BIG + t2
        nc.gpsimd.iota(iobig[:], pattern=[[1, NT]], base=int(BIG),
                       channel_multiplier=0, allow_small_or_imprecise_dtypes=True)
        fbase = const.tile([127, 2], F32, tag="fbase")     # BIG + 254*p + 127*254*h
        nc.gpsimd.iota(fbase[:], pattern=[[127 * 254, 2]], base=int(BIG),
                       channel_multiplier=254, allow_small_or_imprecise_dtypes=True)
        ones1 = const.tile([1, 128], F32, tag="ones1")     # bcast weights
        nc.vector.memset(ones1[:], 1.0)
        ones128 = const.tile([128, 1], F32, tag="ones128")  # reduce weights
        nc.vector.memset(ones128[:], 1.0)
        ones128b = const.tile([128, 1], BF16, tag="ones128b")
        nc.vector.memset(ones128b[:], 1.0)

        # exact threshold table T[t] = fl((t+1)/255), t = 0..253 (Markstein)
        c255 = const.tile([1, 1], F32, tag="c255")
        nc.vector.memset(c255[:], 255.0)
        r255 = const.tile([1, 1], F32, tag="r255")
        nc.vector.reciprocal(r255[:], c255[:])
        iok = const.tile([1, NT], F32, tag="iok")          # 1..254
        nc.gpsimd.iota(iok[:], pattern=[[1, NT]], base=1, channel_multiplier=0,
                       allow_small_or_imprecise_dtypes=True)
        Ttab = const.tile([1, NT], F32, tag="Ttab")
        tA = const.tile([1, NT], F32, tag="tA")
        tS = const.tile([1, NT], F32, tag="tS")
        tD = const.tile([1, NT], F32, tag="tD")
        nc.vector.tensor_scalar(Ttab[:], iok[:], r255[:], None, OP.mult)
        nc.vector.tensor_scalar(tA[:], Ttab[:], 256.0, None, OP.mult)
        nc.vector.tensor_tensor(tS[:], tA[:], Ttab[:], OP.subtract)
        nc.vector.tensor_tensor(tD[:], tA[:], tS[:], OP.subtract)
        nc.vector.tensor_tensor(tD[:], tD[:], Ttab[:], OP.subtract)
        nc.vector.tensor_tensor(tS[:], iok[:], tS[:], OP.subtract)
        nc.vector.tensor_tensor(tS[:], tS[:], tD[:], OP.subtract)
        nc.vector.tensor_scalar(tS[:], tS[:], r255[:], None, OP.mult)
        nc.vector.tensor_tensor(Ttab[:], Ttab[:], tS[:], OP.add)

        bias_tiles = {}

        def bias_ap(val, p=128):
            v = float(np.float32(val))
            if v not in bias_tiles:
                t = const.tile([128, 1], F32, tag=f"bias{len(bias_tiles)}")
                nc.vector.memset(t[:], v)
                bias_tiles[v] = t
            return bias_tiles[v][0:p, :]

        stats = stat_pool.tile([128, 64], F32, tag="stats")
        dbg_row = stat_pool.tile([1, 16], F32, tag="dbg_row")
        nc.vector.memset(stats[:], 0.0)
        nc.vector.memset(dbg_row[:], 0.0)

        def front(b, ds):
            # ---------------- load ----------------
            lab = lab_pool.tile([128, 4 * W], F32, tag="lab")
            nc.sync.dma_start(out=lab[:].rearrange("p (s c) -> p s c", s=4),
                              in_=_sample_view(lab_d, b))
            img = img_pool.tile([128, 4 * W], F32, tag="img")
            nc.sync.dma_start(out=img[:].rearrange("p (s c) -> p s c", s=4),
                              in_=_sample_view(img_d, b))
            prd = prd_pool.tile([128, 4 * W], F32, tag="prd")
            nc.sync.dma_start(out=prd[:].rearrange("p (s c) -> p s c", s=4),
                              in_=_sample_view(prd_d, b))

            M = m_pool.tile([128, 4 * W], F32, tag="M")
            nmp = spsum.tile([1, 512], F32, tag="nmp")
            w = w_pool.tile([128, 4 * W], F32, tag="w")

            # ---------------- dilation + mask ----------------
            labb = lab_pool.tile([128, 4 * W], BF16, tag="labb")
            for s in range(NSLAB):
                nc.scalar.activation(labb[:, 512 * s:512 * (s + 1)],
                                     lab[:, 512 * s:512 * (s + 1)], ACT.Copy)
            for s in range(NSLAB):
                sl = slice(512 * s, 512 * (s + 1))
                yv = vpsum.tile([128, W], F32, tag="yv")
                mms = [(bv_band, s)]
                if s > 0:
                    mms.append((up_band, s - 1))
                if s < NSLAB - 1:
                    mms.append((dn_band, s + 1))
                for i, (band, src) in enumerate(mms):
                    nc.tensor.matmul(
                        yv[:], band[:],
                        labb[:, 512 * src:512 * (src + 1)],
                        start=(i == 0), stop=(i == len(mms) - 1))

                cp = scr_pool.tile([128, 520], F32, tag="cp")
                se = _eng(nc, SCAN_ENG[s])
                nc.vector.memset(cp[:, 0:3], 0.0)
                se.tensor_tensor_scan(
                    cp[:, 3:515], yv[:], labb[:, sl], 0.0, OP.add, OP.bypass)
                nc.vector.tensor_copy(out=cp[:, 515:516], in_=cp[:, 514:515])
                nc.vector.tensor_copy(out=cp[:, 516:517], in_=cp[:, 514:515])
                nc.vector.scalar_tensor_tensor(
                    M[:, sl], cp[:, 5:517], 0.0, cp[:, 0:512],
                    OP.add, OP.is_gt)
                nc.tensor.matmul(nmp[0:1, :], ones128[:], M[:, sl],
                                 start=(s == 0), stop=(s == NSLAB - 1))

            yield
            # ---------------- w + spp ----------------
            # w = (K1*M)*img  (zero where unmasked), per slab for pipelining
            for s in range(NSLAB):
                sl = slice(512 * s, 512 * (s + 1))
                nc.vector.scalar_tensor_tensor(w[:, sl], M[:, sl], K1,
                                                img[:, sl], OP.mult, OP.mult)
            # spp = sum((prd*M)^2): Pool multiply + ACT square-accumulate
            pm = ge_pool.tile([128, 4 * W], F32, tag="pm")
            for s in range(NSLAB):
                sl = slice(512 * s, 512 * (s + 1))
                nc.gpsimd.tensor_tensor(pm[:, sl], prd[:, sl], M[:, sl], OP.mult)
            nc.scalar.activation(pm[:], pm[:], ACT.Square,
                                 accum_out=stats[:, SPP0 + b:SPP0 + b + 1])

            yield
            # ---------------- bin index + planes + hist ----------------
            hist = hpsum.tile([128, 128], F32, tag="hist")
            for s in range(NSLAB):
                sl = slice(512 * s, 512 * (s + 1))
                idxb = scr_pool.tile([128, W], BF16, tag="idxb")
                nc.vector.tensor_scalar(idxb[:], w[:, sl], MAGICM05, MAGIC,
                                        OP.add, OP.subtract)
                tq = scra_pool.tile([128, W], F32, tag="tq")
                nc.vector.tensor_scalar(tq[:], w[:, sl], 0.0625, 15.5,
                                        OP.mult, OP.add)
                h16b = scr_pool.tile([128, W], BF16, tag="h16b")
                nc.vector.tensor_scalar(h16b[:], tq[:], MAGIC, MAGIC,
                                        OP.add, OP.subtract)
                lob = scr_pool.tile([128, W], BF16, tag="lob")
                nc.vector.scalar_tensor_tensor(lob[:], h16b[:], -16.0, idxb[:],
                                               OP.mult, OP.add)

                # planes in [g=64][j=16][k=8] layout: packed last dim keeps
                # the DVE 4x mode, matmul group slices stay contiguous
                A = plA_pool.tile([128, 16 * W], BF16, tag="A")
                Bp = plB_pool.tile([128, 16 * W], BF16, tag="B")
                # A is [g][k][j] (k-outer: strided writes, DVE 2x mode) so the
                # PSUM diagonal lands on contiguous partition blocks; B is
                # [g][j][k] (packed writes, DVE 4x mode)
                Av = A[:].rearrange("p (g k j) -> p j g k", g=64, j=16)
                Bv = Bp[:].rearrange("p (g j k) -> p j g k", g=64, j=16)
                h16g = h16b[:].rearrange("p (g k) -> p g k", g=64)
                lobg = lob[:].rearrange("p (g k) -> p g k", g=64)
                bump = scra_pool.tile([128, W], F32, tag="bump")
                bumpg = bump[:].rearrange("p (g k) -> p g k", g=64)
                for j in range(16):
                    eng = PLANE_ENG[j]
                    jv = float(16 + j)
                    if eng == "dve":
                        nc.vector.tensor_scalar(Av[:, j], h16g, jv, None, OP.is_equal)
                    elif eng == "pool":
                        nc.gpsimd.tensor_scalar(Av[:, j], h16g, jv, None, OP.is_equal)
                    else:
                        nc.scalar.activation(bump[:], h16b[:], ACT.Square, bias=bias_ap(-jv))
                        nc.scalar.activation(Av[:, j], bumpg, ACT.Relu, scale=-1.0, bias=1.0)
                for j in range(16):
                    eng = PLANE_ENG[16 + j]
                    jv = float(j - 256)
                    if eng == "dve":
                        nc.vector.tensor_scalar(Bv[:, j], lobg, jv, None, OP.is_equal)
                    elif eng == "pool":
                        nc.gpsimd.tensor_scalar(Bv[:, j], lobg, jv, None, OP.is_equal)
                    else:
                        nc.scalar.activation(bump[:], lob[:], ACT.Square, bias=bias_ap(-jv))
                        nc.scalar.activation(Bv[:, j], bumpg, ACT.Relu, scale=-1.0, bias=1.0)

                for g in range(W // 8):
                    nc.tensor.matmul(
                        hist[:], A[:, 128 * g:128 * g + 128], Bp[:, 128 * g:128 * g + 128],
                        start=(s == 0 and g == 0),
                        stop=(s == NSLAB - 1 and g == W // 8 - 1))
                yield

            # nm = masked-pixel count (exact) from the PE column-sum psum
            nm = otsu_pool.tile([1, 1], F32, tag=f"nm{b}")
            nc.vector.tensor_reduce(nm[:], nmp[0:1, :], AX.X, OP.add)
            nc.vector.tensor_copy(out=stats[0:1, SM0 + b:SM0 + b + 1], in_=nm[:])
            ds[b] = dict(w=w, prd=prd, hist=hist, nm=nm)

        def back(b, ds):
            yield
            d = ds[b]
            w, prd, hist, nm = d["w"], d["prd"], d["hist"], d["nm"]
            # ---------------- hist assembly ----------------
            # gather the 8 diagonal [16,16] blocks onto partitions 0..15,
            # then tree-sum
            hsb = otsu_pool.tile([128, 128], F32, tag="hsb")
            nc.vector.tensor_copy(out=hsb[:], in_=hist[:])
            hd = otsu_pool.tile([16, 128], F32, tag="hd")
            for k in range(8):
                hv = hsb[16 * k:16 * k + 16, :].rearrange(
                    "p (b kk) -> p kk b", kk=8)
                nc.sync.dma_start(out=hd[:, 16 * k:16 * k + 16],
                                  in_=hv[:, k:k + 1, :])
            hq = otsu_pool.tile([16, 64], F32, tag="hq")
            nc.vector.tensor_tensor(hq[:], hd[:, 0:64], hd[:, 64:128], OP.add)
            nc.vector.tensor_tensor(hq[:, 0:32], hq[:, 0:32], hq[:, 32:64], OP.add)
            hist_s = otsu_pool.tile([16, 16], F32, tag="hist_s")
            nc.vector.tensor_tensor(hist_s[:], hq[:, 0:16], hq[:, 16:32], OP.add)
            hrow = otsu_pool.tile([1, 256], F32, tag="hrow")
            nc.sync.dma_start(out=hrow[:], in_=hist_s[:])

            spt = spsum.tile([128, 8], F32, tag="spt")
            # fix bin (0,0): subtract unmasked count 262144 - nm
            nmm = otsu_pool.tile([1, 1], F32, tag="nmm")
            nc.vector.tensor_scalar(nmm[:], nm[:], -NPIX, None, OP.add)
            nc.vector.tensor_scalar(hrow[0:1, 0:1], hrow[0:1, 0:1], nmm[:],
                                    None, OP.add)

            # ---------------- Otsu ----------------
            rn = otsu_pool.tile([1, 1], F32, tag="rn")
            nc.vector.reciprocal(rn[:], nm[:])
            # brow packs [ch 0:256 | cm 256:512 | r2 512:766 | vw2 766:1020]
            brow = otsu_pool.tile([1, 1020], F32, tag="brow")
            hn = otsu_pool.tile([1, 256], F32, tag="hn256")
            nc.vector.tensor_scalar(hn[:], hrow[:], rn[:], None, OP.mult)
            nc.vector.tensor_tensor_scan(brow[0:1, 0:256], hn[:], hn[:],
                                         0.0, OP.add, OP.bypass)
            hj = otsu_pool.tile([1, 256], F32, tag="hrow")
            nc.vector.tensor_tensor(hj[:], hn[:], io256[:], OP.mult)
            nc.vector.tensor_tensor_scan(brow[0:1, 256:512], hj[:], hj[:],
                                         0.0, OP.add, OP.bypass)
            ch = brow[0:1, 0:256]
            cm = brow[0:1, 256:512]
            tm_ap = brow[0:1, 511:512]     # cm[255]

            yield
            # row terms over t2: r2 = (tm-cm)^2/(1-ch+eps), vw2 = [ch < 1]
            w2p = otsu_pool.tile([1, NT], F32, tag="w2p")
            nc.vector.tensor_scalar(w2p[:], ch[0:1, 0:NT], -1.0, 1.0 + EPS,
                                    OP.mult, OP.add)
            r2r = otsu_pool.tile([1, NT], F32, tag="r2r")
            nc.vector.reciprocal(r2r[:], w2p[:])
            d2 = otsu_pool.tile([1, NT], F32, tag="w2p")
            nc.vector.tensor_scalar(d2[:], cm[0:1, 0:NT], -1.0, tm_ap,
                                    OP.mult, OP.add)
            nc.vector.tensor_tensor(d2[:], d2[:], d2[:], OP.mult)
            nc.vector.tensor_tensor(brow[0:1, 512:766], d2[:], r2r[:], OP.mult)
            nc.vector.tensor_scalar(brow[0:1, 766:1020], ch[0:1, 0:NT], 1.0,
                                    None, OP.is_lt)

            # broadcast rows to 127 partitions via exact f32 PE ones-matmuls
            bps1 = bpsum.tile([127, 512], F32, tag="bps1")
            nc.tensor.matmul(bps1[:], ones1[0:1, 0:127], brow[0:1, 0:512],
                             start=True, stop=True)
            bps2 = bpsum.tile([127, 508], F32, tag="bps2")
            nc.tensor.matmul(bps2[:], ones1[0:1, 0:127], brow[0:1, 512:1020],
                             start=True, stop=True)
            r2B = bps2[:, 0:NT]
            vw2B = bps2[:, NT:2 * NT]
            tmcol = bps1[:, 511:512]

            # per-t1 columns
            acol = otsu_pool.tile([127, 2], F32, tag="acol")
            bcol = otsu_pool.tile([127, 2], F32, tag="bcol")
            for hh in range(2):
                rs = slice(127 * hh, 127 * hh + 127)
                nc.sync.dma_start(out=acol[:, hh:hh + 1], in_=ch[0:1, rs])
                nc.sync.dma_start(out=bcol[:, hh:hh + 1], in_=cm[0:1, rs])
            w0p = otsu_pool.tile([127, 2], F32, tag="w0p")
            nc.vector.tensor_scalar(w0p[:], acol[:], EPS, None, OP.add)
            r0c = otsu_pool.tile([127, 2], F32, tag="r0c")
            nc.vector.reciprocal(r0c[:], w0p[:])
            bv0 = otsu_pool.tile([127, 2], F32, tag="bv0")
            nc.vector.tensor_tensor(bv0[:], bcol[:], bcol[:], OP.mult)
            nc.vector.tensor_tensor(bv0[:], bv0[:], r0c[:], OP.mult)
            vw0 = otsu_pool.tile([127, 2], F32, tag="vw0")
            nc.vector.tensor_scalar(vw0[:], acol[:], 0.0, None, OP.is_gt)

            yield
            colmax2 = otsu_pool.tile([127, 2], F32, tag="colmax2")
            t2min2 = otsu_pool.tile([127, 2], F32, tag="t2min2")
            for hh in range(2):
                a_c = acol[:, hh:hh + 1]
                b_c = bcol[:, hh:hh + 1]
                w1e = otsu_pool.tile([127, NT], F32, tag="w1e")
                nc.vector.tensor_scalar(w1e[:], bps1[:, 0:NT], a_c, EPS,
                                        OP.subtract, OP.add)
                rw1 = otsu_pool.tile([127, NT], F32, tag="rw1")
                nc.vector.reciprocal(rw1[:], w1e[:])
                num = otsu_pool.tile([127, NT], F32, tag="num")
                nc.vector.tensor_scalar(num[:], bps1[:, 256:256 + NT], b_c,
                                        None, OP.subtract)
                nsq = otsu_pool.tile([127, NT], F32, tag="rw1s")
                nc.scalar.activation(nsq[:], num[:], ACT.Square)
                bv = otsu_pool.tile([127, NT], F32, tag="bv")
                nc.gpsimd.tensor_tensor(bv[:], nsq[:], rw1[:], OP.mult)
                # + per-t1 term via ACT bias pointer
                nc.scalar.activation(bv[:], bv[:], ACT.Relu,
                                     bias=bv0[:, hh:hh + 1])
                nc.vector.tensor_tensor(bv[:], bv[:], r2B, OP.add)
                vw1 = otsu_pool.tile([127, NT], F32, tag="rw1s")
                nc.gpsimd.tensor_scalar(vw1[:], w1e[:], EPS, None, OP.is_gt)
                nc.gpsimd.tensor_tensor(bv[:], bv[:], vw1[:], OP.mult)
                nc.vector.tensor_tensor(bv[:], bv[:], vw2B, OP.mult)
                # * per-t1 validity via ACT scale pointer
                nc.scalar.activation(bv[:], bv[:], ACT.Relu,
                                     scale=vw0[:, hh:hh + 1])
                # first-max + index in one op
                mx8 = otsu_pool.tile([127, 8], F32, tag="mx8")
                mi8 = otsu_pool.tile([127, 8], mybir.dt.uint32, tag="mi8")
                nc.vector.max_with_indices(mx8[:], mi8[:], bv[:])
                nc.vector.tensor_copy(out=colmax2[:, hh:hh + 1], in_=mx8[:, 0:1])
                nc.vector.tensor_copy(out=t2min2[:, hh:hh + 1],
                                      in_=mi8[:, 0:1].bitcast(mybir.dt.int32))
                yield

            # global first-max via DMA transpose to one partition
            flat2 = otsu_pool.tile([127, 2], F32, tag="flat2")
            nc.vector.tensor_tensor(flat2[:], t2min2[:], fbase[:], OP.add)
            # natural DMA order interleaves (p,hh) identically for both rows,
            # so elementwise compare/min over the 254 positions stays paired
            grow = otsu_pool.tile([1, 508], F32, tag="w2p")
            nc.sync.dma_start(out=grow[0:1, 0:254], in_=colmax2[:])
            nc.sync.dma_start(out=grow[0:1, 254:508], in_=flat2[:])
            gm = otsu_pool.tile([1, 1], F32, tag="gm")
            nc.vector.tensor_reduce(gm[:], grow[0:1, 0:254], AX.X, OP.max)
            eqr = otsu_pool.tile([1, 254], F32, tag="eqr")
            nc.vector.tensor_scalar(eqr[:], grow[0:1, 0:254], gm[:], None,
                                    OP.is_equal)
            nc.vector.scalar_tensor_tensor(eqr[:], eqr[:], -BIG,
                                           grow[0:1, 254:508], OP.mult, OP.add)
            am = otsu_pool.tile([1, 1], F32, tag="am")
            nc.vector.tensor_reduce(am[:], eqr[:], AX.X, OP.min)

            # t1i = floor((am+0.5)/254) via +1-shifted magic floor
            qt = otsu_pool.tile([1, 1], F32, tag="qt")
            nc.vector.tensor_scalar(qt[:], am[:], R254,
                                    float(np.float32(0.5 * R254 + 1.0)),
                                    OP.mult, OP.add)
            t1p = otsu_pool.tile([1, 1], F32, tag="t1p")
            nc.vector.tensor_scalar(t1p[:], qt[:], MAGICM05, MAGIC,
                                    OP.add, OP.subtract)
            t1i = otsu_pool.tile([1, 1], F32, tag="t1i")
            nc.vector.tensor_scalar(t1i[:], t1p[:], 1.0, None, OP.subtract)
            t2i = otsu_pool.tile([1, 1], F32, tag="t2i")
            nc.vector.scalar_tensor_tensor(t2i[:], t1i[:], -254.0, am[:],
                                           OP.mult, OP.add)

            # threshold lookup + K1 scale, broadcast to 128 partitions
            T12 = otsu_pool.tile([1, 2], F32, tag="T12")
            selv = otsu_pool.tile([1, NT], F32, tag="selv")
            sdump = otsu_pool.tile([1, NT], F32, tag="eqr")
            nc.vector.tensor_scalar(selv[:], iot[:], t1i[:], None, OP.is_equal)
            nc.vector.tensor_tensor(sdump[:], selv[:], Ttab[:], OP.mult)
            nc.vector.tensor_reduce(T12[0:1, 0:1], sdump[:], AX.X, OP.add)
            nc.vector.tensor_scalar(selv[:], iot[:], t2i[:], None, OP.is_equal)
            nc.vector.tensor_tensor(sdump[:], selv[:], Ttab[:], OP.mult)
            nc.vector.tensor_reduce(T12[0:1, 1:2], sdump[:], AX.X, OP.add)
            nc.tensor.matmul(spt[:, 4:6], ones1[:], T12[:], start=True,
                             stop=True, skip_group_check=True)
            W12 = otsu_pool.tile([128, 2], F32, tag="W12")
            nc.vector.tensor_scalar(W12[:], spt[:, 4:6], K1, None, OP.mult)

            nc.vector.tensor_copy(out=dbg_row[:, 4 * b:4 * b + 1], in_=am[:])
            nc.vector.tensor_copy(out=dbg_row[:, 4 * b + 1:4 * b + 2], in_=nm[:])
            nc.vector.tensor_copy(out=dbg_row[:, 4 * b + 2:4 * b + 3], in_=T12[0:1, 0:1])
            nc.vector.tensor_copy(out=dbg_row[:, 4 * b + 3:4 * b + 4], in_=T12[0:1, 1:2])

            yield
            # ---------------- MSE ----------------
            ge1 = ge_pool.tile([128, 4 * W], BF16, tag="ge1")
            nc.vector.tensor_scalar(ge1[:], w[:], W12[:, 0:1], 0.0, OP.is_ge,
                                    OP.add,
                                    accum_out=stats[:, N1C + b:N1C + b + 1])
            gst = ge_pool.tile([128, 4 * W], BF16, tag="gst")
            nc.vector.scalar_tensor_tensor(
                gst[:], w[:], W12[:, 1:2], ge1[:], OP.is_ge, OP.add,
                accum_out=stats[:, N12C + b:N12C + b + 1])
            yield
            for s in range(NSLAB):
                sl = slice(512 * s, 512 * (s + 1))
                sgo = ge_pool.tile([128, W], F32, tag="sgo")
                nc.vector.scalar_tensor_tensor(
                    sgo[:], gst[:, sl], 1.0, prd[:, sl], OP.mult, OP.mult,
                    accum_out=stats[:, SG0 + 4 * b + s:SG0 + 4 * b + s + 1])

        def drain(g):
            for _ in g:
                pass

        def pump(g):
            try:
                next(g)
                return True
            except StopIteration:
                return False

        def interleave(g1, g2):
            alive1 = alive2 = True
            while alive1 or alive2:
                if alive1:
                    alive1 = pump(g1)
                if alive2:
                    alive2 = pump(g2)

        ds = {}
        fg = [front(b, ds) for b in range(B_PER_CORE)]
        bg = [back(b, ds) for b in range(B_PER_CORE)]
        for b in range(B_PER_CORE):
            drain(fg[b])
            drain(bg[b])

        # ---------------- ship stats ----------------
        # exact partition reduction via PE ones-matmul (1.0 * x products exact)
        sred = spsum.tile([64, 1], F32, tag="sred")
        nc.tensor.matmul(sred[:], stats[:], ones128[:], start=True, stop=True)
        srs = stat_pool.tile([64, 1], F32, tag="srs")
        nc.vector.tensor_copy(out=srs[:], in_=sred[:])
        nc.sync.dma_start(out=out_d[:], in_=srs[:])
        nc.sync.dma_start(out=dbg_d[:], in_=dbg_row[:])


_NC_CACHE = None


def _get_nc():
    global _NC_CACHE
    if _NC_CACHE is None:
        _NC_CACHE = build_nc()
    return _NC_CACHE


def kernel(preds, labels, images):
    preds = np.asarray(preds)
    labels = np.asarray(labels)
    images = np.asarray(images)
    B = preds.shape[0]
    assert B == 32 and preds.shape == (32, 1, 512, 512)
    nc = _get_nc()

    in_maps = []
    for c in range(8):
        sl = slice(B_PER_CORE * c, B_PER_CORE * (c + 1))
        in_maps.append({
            "labels": labels[sl, 0].reshape(B_PER_CORE * H, W),
            "images": images[sl, 0].reshape(B_PER_CORE * H, W),
            "preds": preds[sl, 0].reshape(B_PER_CORE * H, W),
        })
    res = run_bass_kernel_spmd(nc, in_maps, list(range(8)))

    sq = np.zeros(32, np.float32)
    sm = np.zeros(32, np.float32)
    for c in range(8):
        st = res.results[c]["stats"][:, 0]
        for b in range(B_PER_CORE):
            i = B_PER_CORE * c + b
            smb = np.float32(st[SM0 + b])
            spp = np.float32(st[SPP0 + b])
            sg = np.sum(st[SG0 + 4 * b:SG0 + 4 * b + 4], dtype=np.float32)
            n1 = np.float32(st[N1C + b])
            n12 = np.float32(st[N12C + b])
            sm[i] = smb
            sq[i] = np.float32(0.75) * n12 - np.float32(0.5) * n1 - sg + spp
    smp = (sm + np.float32(EPS)).astype(np.float32)
    valid = smp > np.float32(1e-8)
    loss_per = (sq / smp).astype(np.float32)
    cnt = np.float32(valid.sum())
    if cnt > 0:
        total = np.sum(np.where(valid, loss_per, np.float32(0.0)), dtype=np.float32)
        out = np.float32(total / np.maximum(cnt, np.float32(1.0)))
    else:
        out = np.float32(0.0)
    return np.float32(out)


# revision 9
# speedup vs baseline: 1.0131x; 1.0131x over previous
"""Trainium2 Bass kernel v2 for nn_Detail_loss (histogram_binning).

Data-parallel over B=32 samples -> 8 cores x 4 samples. Per core/sample:
  1. 5x5 dilation: vertical via PE banded matmuls in f32r (labels are 0/1 so
     f32r is exact), horizontal via row-cumsum difference (scan).
  2. w = 255*(256/255)*M*img in one stt; idx/h16/lo via magic-constant
     floor tricks, all bf16 where possible (DVE 4x mode). Unmasked pixels
     fall in bin (0,0); fixed by subtracting (262144 - nmask) later.
  3. Histogram via 16x16 hi/lo one-hot bf16 planes and 8-column-grouped
     PE outer products into one [128,128] PSUM tile (diagonal 16x16 blocks
     summed afterwards).
  4. Otsu on the simplified criterion bv = m0^2/(w0+e) + (m1-m0)^2/(w1+e)
     + (tm-m1)^2/(w2+e) (same argmax as reference). Row terms broadcast to
     127 partitions via exact f32 PE ones-matmuls (no gpsimd broadcasts).
     First-max row-major tie-break via the -BIG flat-index encoding.
  5. MSE via the expansion sq = 0.75*N12 - 0.5*N1 - SG + SPP with
     ge-compares on w against W_k = K1*T_k (exact modulo ~ulp windows),
     N's from accum_out, SG = sum((ge1+ge2)*prd) via tensor_tensor_reduce,
     SPP = sum(prd^2*M) precomputed in the front-end.
Host: loss = mean over valid samples of sq/sm (np.float32 math).
"""

import os

import numpy as np

import concourse.bass as bass
import concourse.mybir as mybir
from concourse import bacc, bass_isa, tile
from concourse.bass_utils import run_bass_kernel_spmd

F32 = mybir.dt.float32
F32R = mybir.dt.float32r
BF16 = mybir.dt.bfloat16
OP = mybir.AluOpType
ACT = mybir.ActivationFunctionType
AX = mybir.AxisListType

B_PER_CORE = 4
H = 512
W = 512
NSLAB = 4
SLABW = 512
NBINS = 256
NT = 254
BIG = 4194304.0      # 2^22
MAGIC = 8388608.0    # 2^23
MAGICM05 = 8388607.5
EPS = 1e-8
NPIX = 262144.0      # 512*512

C_BIN = float(np.float32(256.0 / 255.0))
K1 = float(np.float32(255.0) * np.float32(256.0 / 255.0))  # w = K1*img*M
R254 = float(np.float32(1.0) / np.float32(254.0))

# engine per one-hot plane (32 = 16 hi then 16 lo)
PLANE_ENG = (["dve"] * 9 + ["pool"] * 4 + ["act"] * 3 +
             ["dve"] * 11 + ["pool"] * 4 + ["act"] * 1)
assert len(PLANE_ENG) == 32
SCAN_ENG = ["dve", "dve", "dve", "dve"]  # scan reads PSUM; gpsimd cannot
M_ENG = ["dve", "dve", "dve", "dve"]  # accum_out unsupported on Pool

# stats columns in [128, 64]
SM0 = 0     # 0..15   sm(b,s)
SPP0 = 16   # 16..31  spp(b,s)
SG0 = 32    # 32..47  sg(b,s)
N1C = 48    # 48..51  N1(b)
N12C = 52   # 52..55  N12(b)


def build_nc():
    nc = bacc.Bacc("TRN2", target_bir_lowering=False)

    lab_d = nc.dram_tensor("labels", [B_PER_CORE * H, W], F32, kind="ExternalInput")
    img_d = nc.dram_tensor("images", [B_PER_CORE * H, W], F32, kind="ExternalInput")
    prd_d = nc.dram_tensor("preds", [B_PER_CORE * H, W], F32, kind="ExternalInput")
    out_d = nc.dram_tensor("stats", [64, 1], F32, kind="ExternalOutput")
    dbg_d = nc.dram_tensor("dbg", [1, 16], F32, kind="ExternalOutput")

    with tile.TileContext(nc) as tc:
        _emit(nc, tc, lab_d, img_d, prd_d, out_d, dbg_d)
    nc.compile()
    return nc


def _sample_view(dram, b):
    return dram[512 * b:512 * (b + 1), :].rearrange("(s p) c -> p s c", p=128)


def _eng(nc, name):
    return {"dve": nc.vector, "pool": nc.gpsimd, "act": nc.scalar}[name]


def _emit(nc, tc, lab_d, img_d, prd_d, out_d, dbg_d):
    import contextlib
    ctx = contextlib.ExitStack()
    with ctx:
        const = ctx.enter_context(tc.tile_pool(name="const", bufs=1))
        lab_pool = ctx.enter_context(tc.tile_pool(name="lab", bufs=1))
        img_pool = ctx.enter_context(tc.tile_pool(name="img", bufs=2))
        prd_pool = ctx.enter_context(tc.tile_pool(name="prd", bufs=2))
        m_pool = ctx.enter_context(tc.tile_pool(name="mask", bufs=2))
        w_pool = ctx.enter_context(tc.tile_pool(name="w", bufs=2))
        scr_pool = ctx.enter_context(tc.tile_pool(name="scr", bufs=3))
        scra_pool = ctx.enter_context(tc.tile_pool(name="scra", bufs=1))
        ge_pool = ctx.enter_context(tc.tile_pool(name="ge", bufs=1))
        plA_pool = ctx.enter_context(tc.tile_pool(name="plA", bufs=2))
        plB_pool = ctx.enter_context(tc.tile_pool(name="plB", bufs=2))
        otsu_pool = ctx.enter_context(tc.tile_pool(name="otsu", bufs=1))
        stat_pool = ctx.enter_context(tc.tile_pool(name="stat", bufs=1))
        vpsum = ctx.enter_context(
            tc.tile_pool(name="vpsum", bufs=1, space=bass.MemorySpace.PSUM))
        hpsum = ctx.enter_context(
            tc.tile_pool(name="hpsum", bufs=2, space=bass.MemorySpace.PSUM))
        bpsum = ctx.enter_context(
            tc.tile_pool(name="bpsum", bufs=1, space=bass.MemorySpace.PSUM))
        spsum = ctx.enter_context(
            tc.tile_pool(name="spsum", bufs=1, space=bass.MemorySpace.PSUM))

        # ---------------- constants ----------------
        io_fp = const.tile([128, 128], mybir.dt.int32, tag="io_fp")   # f - p
        nc.gpsimd.iota(io_fp[:], pattern=[[1, 128]], base=0, channel_multiplier=-1)
        io_pf = const.tile([128, 128], mybir.dt.int32, tag="io_pf")   # p - f
        nc.gpsimd.iota(io_pf[:], pattern=[[-1, 128]], base=0, channel_multiplier=1)

        bv_band = const.tile([128, 128], BF16, tag="bv_band")
        btmp = const.tile([128, 128], F32, tag="btmp")
        nc.vector.tensor_scalar(btmp[:], io_fp[:], -2, None, OP.is_ge)
        nc.vector.scalar_tensor_tensor(bv_band[:], io_fp[:], 2, btmp[:], OP.is_le, OP.mult)
        up_band = const.tile([128, 128], BF16, tag="up_band")
        nc.vector.tensor_scalar(up_band[:], io_pf[:], 126, None, OP.is_ge)
        dn_band = const.tile([128, 128], BF16, tag="dn_band")
        nc.vector.tensor_scalar(dn_band[:], io_fp[:], 126, None, OP.is_ge)

        io256 = const.tile([1, 256], F32, tag="io256")     # 0..255
        nc.gpsimd.iota(io256[:], pattern=[[1, 256]], base=0, channel_multiplier=0,
                       allow_small_or_imprecise_dtypes=True)
        iot = const.tile([1, NT], F32, tag="iot")          # 0..253
        nc.gpsimd.iota(iot[:], pattern=[[1, NT]], base=0, channel_multiplier=0,
                       allow_small_or_imprecise_dtypes=True)
        iobig = const.tile([127, NT], F32, tag="iobig")    # BIG + t2
        nc.gpsimd.iota(iobig[:], pattern=[[1, NT]], base=int(BIG),
                       channel_multiplier=0, allow_small_or_imprecise_dtypes=True)
        fbase = const.tile([127, 2], F32, tag="fbase")     # BIG + 254*p + 127*254*h
        nc.gpsimd.iota(fbase[:], pattern=[[127 * 254, 2]], base=int(BIG),
                       channel_multiplier=254, allow_small_or_imprecise_dtypes=True)
        ones1 = const.tile([1, 128], F32, tag="ones1")     # bcast weights
        nc.vector.memset(ones1[:], 1.0)
        ones128 = const.tile([128, 1], F32, tag="ones128")  # reduce weights
        nc.vector.memset(ones128[:], 1.0)
        ones128b = const.tile([128, 1], BF16, tag="ones128b")
        nc.vector.memset(ones128b[:], 1.0)

        # exact threshold table T[t] = fl((t+1)/255), t = 0..253 (Markstein)
        c255 = const.tile([1, 1], F32, tag="c255")
        nc.vector.memset(c255[:], 255.0)
        r255 = const.tile([1, 1], F32, tag="r255")
        nc.vector.reciprocal(r255[:], c255[:])
        iok = const.tile([1, NT], F32, tag="iok")          # 1..254
        nc.gpsimd.iota(iok[:], pattern=[[1, NT]], base=1, channel_multiplier=0,
                       allow_small_or_imprecise_dtypes=True)
        Ttab = const.tile([1, NT], F32, tag="Ttab")
        tA = const.tile([1, NT], F32, tag="tA")
        tS = const.tile([1, NT], F32, tag="tS")
        tD = const.tile([1, NT], F32, tag="tD")
        nc.vector.tensor_scalar(Ttab[:], iok[:], r255[:], None, OP.mult)
        nc.vector.tensor_scalar(tA[:], Ttab[:], 256.0, None, OP.mult)
        nc.vector.tensor_tensor(tS[:], tA[:], Ttab[:], OP.subtract)
        nc.vector.tensor_tensor(tD[:], tA[:], tS[:], OP.subtract)
        nc.vector.tensor_tensor(tD[:], tD[:], Ttab[:], OP.subtract)
        nc.vector.tensor_tensor(tS[:], iok[:], tS[:], OP.subtract)
        nc.vector.tensor_tensor(tS[:], tS[:], tD[:], OP.subtract)
        nc.vector.tensor_scalar(tS[:], tS[:], r255[:], None, OP.mult)
        nc.vector.tensor_tensor(Ttab[:], Ttab[:], tS[:], OP.add)

        bias_tiles = {}

        def bias_ap(val, p=128):
            v = float(np.float32(val))
            if v not in bias_tiles:
                t = const.tile([128, 1], F32, tag=f"bias{len(bias_tiles)}")
                nc.vector.memset(t[:], v)
                bias_tiles[v] = t
            return bias_tiles[v][0:p, :]

        stats = stat_pool.tile([128, 64], F32, tag="stats")
        dbg_row = stat_pool.tile([1, 16], F32, tag="dbg_row")
        nc.vector.memset(stats[:], 0.0)
        nc.vector.memset(dbg_row[:], 0.0)

        def front(b, ds):
            # ---------------- load ----------------
            lab = lab_pool.tile([128, 4 * W], F32, tag="lab")
            nc.sync.dma_start(out=lab[:].rearrange("p (s c) -> p s c", s=4),
                              in_=_sample_view(lab_d, b))
            img = img_pool.tile([128, 4 * W], F32, tag="img")
            nc.sync.dma_start(out=img[:].rearrange("p (s c) -> p s c", s=4),
                              in_=_sample_view(img_d, b))
            prd = prd_pool.tile([128, 4 * W], F32, tag="prd")
            nc.sync.dma_start(out=prd[:].rearrange("p (s c) -> p s c", s=4),
                              in_=_sample_view(prd_d, b))

            M = m_pool.tile([128, 4 * W], F32, tag="M")
            nmp = spsum.tile([1, 512], F32, tag="nmp")
            w = w_pool.tile([128, 4 * W], F32, tag="w")

            # ---------------- dilation + mask ----------------
            labb = lab_pool.tile([128, 4 * W], BF16, tag="labb")
            for s in range(NSLAB):
                nc.scalar.activation(labb[:, 512 * s:512 * (s + 1)],
                                     lab[:, 512 * s:512 * (s + 1)], ACT.Copy)
            for s in range(NSLAB):
                sl = slice(512 * s, 512 * (s + 1))
                yv = vpsum.tile([128, W], F32, tag="yv")
                mms = [(bv_band, s)]
                if s > 0:
                    mms.append((up_band, s - 1))
                if s < NSLAB - 1:
                    mms.append((dn_band, s + 1))
                for i, (band, src) in enumerate(mms):
                    nc.tensor.matmul(
                        yv[:], band[:],
                        labb[:, 512 * src:512 * (src + 1)],
                        start=(i == 0), stop=(i == len(mms) - 1))

                cp = scr_pool.tile([128, 520], F32, tag="cp")
                se = _eng(nc, SCAN_ENG[s])
                nc.vector.memset(cp[:, 0:3], 0.0)
                se.tensor_tensor_scan(
                    cp[:, 3:515], yv[:], labb[:, sl], 0.0, OP.add, OP.bypass)
                nc.vector.tensor_copy(out=cp[:, 515:516], in_=cp[:, 514:515])
                nc.vector.tensor_copy(out=cp[:, 516:517], in_=cp[:, 514:515])
                nc.vector.tensor_tensor(
                    M[:, sl], cp[:, 5:517], cp[:, 0:512], OP.is_gt)
                nc.tensor.matmul(nmp[0:1, :], ones128[:], M[:, sl],
                                 start=(s == 0), stop=(s == NSLAB - 1))

            yield
            # ---------------- w + spp ----------------
            # w = (K1*M)*img  (zero where unmasked), per slab for pipelining
            for s in range(NSLAB):
                sl = slice(512 * s, 512 * (s + 1))
                nc.vector.scalar_tensor_tensor(w[:, sl], M[:, sl], K1,
                                                img[:, sl], OP.mult, OP.mult)
            # spp = sum((prd*M)^2): Pool multiply + ACT square-accumulate
            pm = ge_pool.tile([128, 4 * W], F32, tag="pm")
            for s in range(NSLAB):
                sl = slice(512 * s, 512 * (s + 1))
                nc.gpsimd.tensor_tensor(pm[:, sl], prd[:, sl], M[:, sl], OP.mult)
            nc.scalar.activation(pm[:], pm[:], ACT.Square,
                                 accum_out=stats[:, SPP0 + b:SPP0 + b + 1])

            yield
            # ---------------- bin index + planes + hist ----------------
            hist = hpsum.tile([128, 128], F32, tag="hist")
            for s in range(NSLAB):
                sl = slice(512 * s, 512 * (s + 1))
                idxb = scr_pool.tile([128, W], BF16, tag="idxb")
                nc.vector.tensor_scalar(idxb[:], w[:, sl], MAGICM05, MAGIC,
                                        OP.add, OP.subtract)
                tq = scra_pool.tile([128, W], F32, tag="tq")
                nc.vector.tensor_scalar(tq[:], w[:, sl], 0.0625, 15.5,
                                        OP.mult, OP.add)
                h16b = scr_pool.tile([128, W], BF16, tag="h16b")
                nc.vector.tensor_scalar(h16b[:], tq[:], MAGIC, MAGIC,
                                        OP.add, OP.subtract)
                lob = scr_pool.tile([128, W], BF16, tag="lob")
                nc.vector.scalar_tensor_tensor(lob[:], h16b[:], -16.0, idxb[:],
                                               OP.mult, OP.add)

                # planes in [g=64][j=16][k=8] layout: packed last dim keeps
                # the DVE 4x mode, matmul group slices stay contiguous
                A = plA_pool.tile([128, 16 * W], BF16, tag="A")
                Bp = plB_pool.tile([128, 16 * W], BF16, tag="B")
                # A is [g][k][j] (k-outer: strided writes, DVE 2x mode) so the
                # PSUM diagonal lands on contiguous partition blocks; B is
                # [g][j][k] (packed writes, DVE 4x mode)
                Av = A[:].rearrange("p (g k j) -> p j g k", g=64, j=16)
                Bv = Bp[:].rearrange("p (g j k) -> p j g k", g=64, j=16)
                h16g = h16b[:].rearrange("p (g k) -> p g k", g=64)
                lobg = lob[:].rearrange("p (g k) -> p g k", g=64)
                bump = scra_pool.tile([128, W], F32, tag="bump")
                bumpg = bump[:].rearrange("p (g k) -> p g k", g=64)
                for j in range(16):
                    eng = PLANE_ENG[j]
                    jv = float(16 + j)
                    if eng == "dve":
                        nc.vector.tensor_scalar(Av[:, j], h16g, jv, None, OP.is_equal)
                    elif eng == "pool":
                        nc.gpsimd.tensor_scalar(Av[:, j], h16g, jv, None, OP.is_equal)
                    else:
                        nc.scalar.activation(bump[:], h16b[:], ACT.Square, bias=bias_ap(-jv))
                        nc.scalar.activation(Av[:, j], bumpg, ACT.Relu, scale=-1.0, bias=1.0)
                for j in range(16):
                    eng = PLANE_ENG[16 + j]
                    jv = float(j - 256)
                    if eng == "dve":
                        nc.vector.tensor_scalar(Bv[:, j], lobg, jv, None, OP.is_equal)
                    elif eng == "pool":
                        nc.gpsimd.tensor_scalar(Bv[:, j], lobg, jv, None, OP.is_equal)
                    else:
                        nc.scalar.activation(bump[:], lob[:], ACT.Square, bias=bias_ap(-jv))
                        nc.scalar.activation(Bv[:, j], bumpg, ACT.Relu, scale=-1.0, bias=1.0)

                for g in range(W // 8):
                    nc.tensor.matmul(
                        hist[:], A[:, 128 * g:128 * g + 128], Bp[:, 128 * g:128 * g + 128],
                        start=(s == 0 and g == 0),
                        stop=(s == NSLAB - 1 and g == W // 8 - 1))
                yield

            # nm = masked-pixel count (exact) from the PE column-sum psum
            nm = otsu_pool.tile([1, 1], F32, tag=f"nm{b}")
            nc.vector.tensor_reduce(nm[:], nmp[0:1, :], AX.X, OP.add)
            nc.vector.tensor_copy(out=stats[0:1, SM0 + b:SM0 + b + 1], in_=nm[:])
            ds[b] = dict(w=w, prd=prd, hist=hist, nm=nm)

        def back(b, ds):
            yield
            d = ds[b]
            w, prd, hist, nm = d["w"], d["prd"], d["hist"], d["nm"]
            # ---------------- hist assembly ----------------
            # gather the 8 diagonal [16,16] blocks onto partitions 0..15,
            # then tree-sum
            hsb = otsu_pool.tile([128, 128], F32, tag="hsb")
            nc.vector.tensor_copy(out=hsb[:], in_=hist[:])
            hd = otsu_pool.tile([16, 128], F32, tag="hd")
            for k in range(8):
                hv = hsb[16 * k:16 * k + 16, :].rearrange(
                    "p (b kk) -> p kk b", kk=8)
                nc.sync.dma_start(out=hd[:, 16 * k:16 * k + 16],
                                  in_=hv[:, k:k + 1, :])
            hq = otsu_pool.tile([16, 64], F32, tag="hq")
            nc.vector.tensor_tensor(hq[:], hd[:, 0:64], hd[:, 64:128], OP.add)
            nc.vector.tensor_tensor(hq[:, 0:32], hq[:, 0:32], hq[:, 32:64], OP.add)
            hist_s = otsu_pool.tile([16, 16], F32, tag="hist_s")
            nc.vector.tensor_tensor(hist_s[:], hq[:, 0:16], hq[:, 16:32], OP.add)
            hrow = otsu_pool.tile([1, 256], F32, tag="hrow")
            nc.sync.dma_start(out=hrow[:], in_=hist_s[:])

            spt = spsum.tile([128, 8], F32, tag="spt")
            # fix bin (0,0): subtract unmasked count 262144 - nm
            nmm = otsu_pool.tile([1, 1], F32, tag="nmm")
            nc.vector.tensor_scalar(nmm[:], nm[:], -NPIX, None, OP.add)
            nc.vector.tensor_scalar(hrow[0:1, 0:1], hrow[0:1, 0:1], nmm[:],
                                    None, OP.add)

            # ---------------- Otsu ----------------
            rn = otsu_pool.tile([1, 1], F32, tag="rn")
            nc.vector.reciprocal(rn[:], nm[:])
            # brow packs [ch 0:256 | cm 256:512 | r2 512:766 | vw2 766:1020]
            brow = otsu_pool.tile([1, 1020], F32, tag="brow")
            hn = otsu_pool.tile([1, 256], F32, tag="hn256")
            nc.vector.tensor_scalar(hn[:], hrow[:], rn[:], None, OP.mult)
            nc.vector.tensor_tensor_scan(brow[0:1, 0:256], hn[:], hn[:],
                                         0.0, OP.add, OP.bypass)
            hj = otsu_pool.tile([1, 256], F32, tag="hrow")
            nc.vector.tensor_tensor(hj[:], hn[:], io256[:], OP.mult)
            nc.vector.tensor_tensor_scan(brow[0:1, 256:512], hj[:], hj[:],
                                         0.0, OP.add, OP.bypass)
            ch = brow[0:1, 0:256]
            cm = brow[0:1, 256:512]
            tm_ap = brow[0:1, 511:512]     # cm[255]

            yield
            # row terms over t2: r2 = (tm-cm)^2/(1-ch+eps), vw2 = [ch < 1]
            w2p = otsu_pool.tile([1, NT], F32, tag="w2p")
            nc.vector.tensor_scalar(w2p[:], ch[0:1, 0:NT], -1.0, 1.0 + EPS,
                                    OP.mult, OP.add)
            r2r = otsu_pool.tile([1, NT], F32, tag="r2r")
            nc.vector.reciprocal(r2r[:], w2p[:])
            d2 = otsu_pool.tile([1, NT], F32, tag="w2p")
            nc.vector.tensor_scalar(d2[:], cm[0:1, 0:NT], -1.0, tm_ap,
                                    OP.mult, OP.add)
            nc.vector.tensor_tensor(d2[:], d2[:], d2[:], OP.mult)
            nc.vector.tensor_tensor(brow[0:1, 512:766], d2[:], r2r[:], OP.mult)
            nc.vector.tensor_scalar(brow[0:1, 766:1020], ch[0:1, 0:NT], 1.0,
                                    None, OP.is_lt)

            # broadcast rows to 127 partitions via exact f32 PE ones-matmuls
            bps1 = bpsum.tile([127, 512], F32, tag="bps1")
            nc.tensor.matmul(bps1[:], ones1[0:1, 0:127], brow[0:1, 0:512],
                             start=True, stop=True)
            bps2 = bpsum.tile([127, 508], F32, tag="bps2")
            nc.tensor.matmul(bps2[:], ones1[0:1, 0:127], brow[0:1, 512:1020],
                             start=True, stop=True)
            r2B = bps2[:, 0:NT]
            vw2B = bps2[:, NT:2 * NT]
            tmcol = bps1[:, 511:512]

            # per-t1 columns
            acol = otsu_pool.tile([127, 2], F32, tag="acol")
            bcol = otsu_pool.tile([127, 2], F32, tag="bcol")
            for hh in range(2):
                rs = slice(127 * hh, 127 * hh + 127)
                nc.sync.dma_start(out=acol[:, hh:hh + 1], in_=ch[0:1, rs])
                nc.sync.dma_start(out=bcol[:, hh:hh + 1], in_=cm[0:1, rs])
            w0p = otsu_pool.tile([127, 2], F32, tag="w0p")
            nc.vector.tensor_scalar(w0p[:], acol[:], EPS, None, OP.add)
            r0c = otsu_pool.tile([127, 2], F32, tag="r0c")
            nc.vector.reciprocal(r0c[:], w0p[:])
            bv0 = otsu_pool.tile([127, 2], F32, tag="bv0")
            nc.vector.tensor_tensor(bv0[:], bcol[:], bcol[:], OP.mult)
            nc.vector.tensor_tensor(bv0[:], bv0[:], r0c[:], OP.mult)
            vw0 = otsu_pool.tile([127, 2], F32, tag="vw0")
            nc.vector.tensor_scalar(vw0[:], acol[:], 0.0, None, OP.is_gt)

            yield
            colmax2 = otsu_pool.tile([127, 2], F32, tag="colmax2")
            t2min2 = otsu_pool.tile([127, 2], F32, tag="t2min2")
            for hh in range(2):
                a_c = acol[:, hh:hh + 1]
                b_c = bcol[:, hh:hh + 1]
                w1e = otsu_pool.tile([127, NT], F32, tag="w1e")
                nc.vector.tensor_scalar(w1e[:], bps1[:, 0:NT], a_c, EPS,
                                        OP.subtract, OP.add)
                rw1 = otsu_pool.tile([127, NT], F32, tag="rw1")
                nc.vector.reciprocal(rw1[:], w1e[:])
                num = otsu_pool.tile([127, NT], F32, tag="num")
                nc.vector.tensor_scalar(num[:], bps1[:, 256:256 + NT], b_c,
                                        None, OP.subtract)
                nsq = otsu_pool.tile([127, NT], F32, tag="rw1s")
                nc.scalar.activation(nsq[:], num[:], ACT.Square)
                bv = otsu_pool.tile([127, NT], F32, tag="bv")
                nc.gpsimd.tensor_tensor(bv[:], nsq[:], rw1[:], OP.mult)
                # + per-t1 term via ACT bias pointer
                nc.scalar.activation(bv[:], bv[:], ACT.Relu,
                                     bias=bv0[:, hh:hh + 1])
                nc.vector.tensor_tensor(bv[:], bv[:], r2B, OP.add)
                vw1 = otsu_pool.tile([127, NT], F32, tag="rw1s")
                nc.gpsimd.tensor_scalar(vw1[:], w1e[:], EPS, None, OP.is_gt)
                nc.gpsimd.tensor_tensor(bv[:], bv[:], vw1[:], OP.mult)
                nc.vector.tensor_tensor(bv[:], bv[:], vw2B, OP.mult)
                # * per-t1 validity via ACT scale pointer
                nc.scalar.activation(bv[:], bv[:], ACT.Relu,
                                     scale=vw0[:, hh:hh + 1])
                # first-max + index in one op
                mx8 = otsu_pool.tile([127, 8], F32, tag="mx8")
                mi8 = otsu_pool.tile([127, 8], mybir.dt.uint32, tag="mi8")
                nc.vector.max_with_indices(mx8[:], mi8[:], bv[:])
                nc.vector.tensor_copy(out=colmax2[:, hh:hh + 1], in_=mx8[:, 0:1])
                nc.vector.tensor_copy(out=t2min2[:, hh:hh + 1],
                                      in_=mi8[:, 0:1].bitcast(mybir.dt.int32))
                yield

            # global first-max via DMA transpose to one partition
            flat2 = otsu_pool.tile([127, 2], F32, tag="flat2")
            nc.vector.tensor_tensor(flat2[:], t2min2[:], fbase[:], OP.add)
            # natural DMA order interleaves (p,hh) identically for both rows,
            # so elementwise compare/min over the 254 positions stays paired
            grow = otsu_pool.tile([1, 508], F32, tag="w2p")
            nc.sync.dma_start(out=grow[0:1, 0:254], in_=colmax2[:])
            nc.sync.dma_start(out=grow[0:1, 254:508], in_=flat2[:])
            gm = otsu_pool.tile([1, 1], F32, tag="gm")
            nc.vector.tensor_reduce(gm[:], grow[0:1, 0:254], AX.X, OP.max)
            eqr = otsu_pool.tile([1, 254], F32, tag="eqr")
            nc.vector.tensor_scalar(eqr[:], grow[0:1, 0:254], gm[:], None,
                                    OP.is_equal)
            nc.vector.scalar_tensor_tensor(eqr[:], eqr[:], -BIG,
                                           grow[0:1, 254:508], OP.mult, OP.add)
            am = otsu_pool.tile([1, 1], F32, tag="am")
            nc.vector.tensor_reduce(am[:], eqr[:], AX.X, OP.min)

            # t1i = floor((am+0.5)/254) via +1-shifted magic floor
            qt = otsu_pool.tile([1, 1], F32, tag="qt")
            nc.vector.tensor_scalar(qt[:], am[:], R254,
                                    float(np.float32(0.5 * R254 + 1.0)),
                                    OP.mult, OP.add)
            t1p = otsu_pool.tile([1, 1], F32, tag="t1p")
            nc.vector.tensor_scalar(t1p[:], qt[:], MAGICM05, MAGIC,
                                    OP.add, OP.subtract)
            t1i = otsu_pool.tile([1, 1], F32, tag="t1i")
            nc.vector.tensor_scalar(t1i[:], t1p[:], 1.0, None, OP.subtract)
            t2i = otsu_pool.tile([1, 1], F32, tag="t2i")
            nc.vector.scalar_tensor_tensor(t2i[:], t1i[:], -254.0, am[:],
                                           OP.mult, OP.add)

            # threshold lookup + K1 scale, broadcast to 128 partitions
            T12 = otsu_pool.tile([1, 2], F32, tag="T12")
            selv = otsu_pool.tile([1, NT], F32, tag="selv")
            sdump = otsu_pool.tile([1, NT], F32, tag="eqr")
            nc.vector.tensor_scalar(selv[:], iot[:], t1i[:], None, OP.is_equal)
            nc.vector.tensor_tensor(sdump[:], selv[:], Ttab[:], OP.mult)
            nc.vector.tensor_reduce(T12[0:1, 0:1], sdump[:], AX.X, OP.add)
            nc.vector.tensor_scalar(selv[:], iot[:], t2i[:], None, OP.is_equal)
            nc.vector.tensor_tensor(sdump[:], selv[:], Ttab[:], OP.mult)
            nc.vector.tensor_reduce(T12[0:1, 1:2], sdump[:], AX.X, OP.add)
            nc.tensor.matmul(spt[:, 4:6], ones1[:], T12[:], start=True,
                             stop=True, skip_group_check=True)
            W12 = otsu_pool.tile([128, 2], F32, tag="W12")
            nc.vector.tensor_scalar(W12[:], spt[:, 4:6], K1, None, OP.mult)

            nc.vector.tensor_copy(out=dbg_row[:, 4 * b:4 * b + 1], in_=am[:])
            nc.vector.tensor_copy(out=dbg_row[:, 4 * b + 1:4 * b + 2], in_=nm[:])
            nc.vector.tensor_copy(out=dbg_row[:, 4 * b + 2:4 * b + 3], in_=T12[0:1, 0:1])
            nc.vector.tensor_copy(out=dbg_row[:, 4 * b + 3:4 * b + 4], in_=T12[0:1, 1:2])

            yield
            # ---------------- MSE ----------------
            ge1 = ge_pool.tile([128, 4 * W], BF16, tag="ge1")
            nc.vector.tensor_scalar(ge1[:], w[:], W12[:, 0:1], 0.0, OP.is_ge,
                                    OP.add,
                                    accum_out=stats[:, N1C + b:N1C + b + 1])
            gst = ge_pool.tile([128, 4 * W], BF16, tag="gst")
            nc.vector.scalar_tensor_tensor(
                gst[:], w[:], W12[:, 1:2], ge1[:], OP.is_ge, OP.add,
                accum_out=stats[:, N12C + b:N12C + b + 1])
            yield
            for s in range(NSLAB):
                sl = slice(512 * s, 512 * (s + 1))
                sgo = ge_pool.tile([128, W], F32, tag="sgo")
                nc.vector.scalar_tensor_tensor(
                    sgo[:], gst[:, sl], 1.0, prd[:, sl], OP.mult, OP.mult,
                    accum_out=stats[:, SG0 + 4 * b + s:SG0 + 4 * b + s + 1])

        def drain(g):
            for _ in g:
                pass

        def pump(g):
            try:
                next(g)
                return True
            except StopIteration:
                return False

        def interleave(g1, g2):
            alive1 = alive2 = True
            while alive1 or alive2:
                if alive1:
                    alive1 = pump(g1)
                if alive2:
                    alive2 = pump(g2)

        ds = {}
        fg = [front(b, ds) for b in range(B_PER_CORE)]
        bg = [back(b, ds) for b in range(B_PER_CORE)]
        for b in range(B_PER_CORE):
            drain(fg[b])
            drain(bg[b])

        # ---------------- ship stats ----------------
        # exact partition reduction via PE ones-matmul (1.0 * x products exact)
        sred = spsum.tile([64, 1], F32, tag="sred")
        nc.tensor.matmul(sred[:], stats[:], ones128[:], start=True, stop=True)
        srs = stat_pool.tile([64, 1], F32, tag="srs")
        nc.vector.tensor_copy(out=srs[:], in_=sred[:])
        nc.sync.dma_start(out=out_d[:], in_=srs[:])
        nc.sync.dma_start(out=dbg_d[:], in_=dbg_row[:])


_NC_CACHE = None


def _get_nc():
    global _NC_CACHE
    if _NC_CACHE is None:
        _NC_CACHE = build_nc()
    return _NC_CACHE


def kernel(preds, labels, images):
    preds = np.asarray(preds)
    labels = np.asarray(labels)
    images = np.asarray(images)
    B = preds.shape[0]
    assert B == 32 and preds.shape == (32, 1, 512, 512)
    nc = _get_nc()

    in_maps = []
    for c in range(8):
        sl = slice(B_PER_CORE * c, B_PER_CORE * (c + 1))
        in_maps.append({
            "labels": labels[sl, 0].reshape(B_PER_CORE * H, W),
            "images": images[sl, 0].reshape(B_PER_CORE * H, W),
            "preds": preds[sl, 0].reshape(B_PER_CORE * H, W),
        })
    res = run_bass_kernel_spmd(nc, in_maps, list(range(8)))

    sq = np.zeros(32, np.float32)
    sm = np.zeros(32, np.float32)
    for c in range(8):
        st = res.results[c]["stats"][:, 0]
        for b in range(B_PER_CORE):
            i = B_PER_CORE * c + b
            smb = np.float32(st[SM0 + b])
            spp = np.float32(st[SPP0 + b])
            sg = np.sum(st[SG0 + 4 * b:SG0 + 4 * b + 4], dtype=np.float32)
            n1 = np.float32(st[N1C + b])
            n12 = np.float32(st[N12C + b])
            sm[i] = smb
            sq[i] = np.float32(0.75) * n12 - np.float32(0.5) * n1 - sg + spp
    smp = (sm + np.float32(EPS)).astype(np.float32)
    valid = smp > np.float32(1e-8)
    loss_per = (sq / smp).astype(np.float32)
    cnt = np.float32(valid.sum())
    if cnt > 0:
        total = np.sum(np.where(valid, loss_per, np.float32(0.0)), dtype=np.float32)
        out = np.float32(total / np.maximum(cnt, np.float32(1.0)))
    else:
        out = np.float32(0.0)
    return np.float32(out)
